# revision 23
# baseline (speedup 1.0000x reference)
"""Sliding-window attention (WINDOW=129) Trainium2 Bass kernel.

Problem: x[B=2, N=2048, C=768] -> qkv proj -> 12-head sliding-window
attention (half-window 64) -> output proj + bias.

Sharding: sequence-parallel over 8 cores: core c handles batch b = c//4,
query chunk s = c%4 (512 queries), with a 64-row halo each side for K/V.
Weights replicated. Each core computes its 512 output rows completely;
host concatenates. No collectives.

Per-core pipeline (matmul operands fp16, psum f32):
  qkT [e, n] via M=128 matmuls (two 64-row head-groups per psum); the
    upper psum half moves to partition base 0 via stream_shuffle (matmul
    operands at partition base 64 fault this device) + ACT copy.
  v -> vaug [n, 12*(64+1)] with a fused ones column per head.
  scores sT[k, q] per key-tile via K=64 matmuls; exp on ACT (scores are
    N(0,1)-scale so no max subtraction); 0/1 band/validity mask on DVE.
  AV with q on the output partition dim: out[q, 65] = pT_slice.T @ vaug;
    column 64 gives the softmax denominator; reciprocal + per-partition
    broadcast multiply normalizes; PE transpose -> attnT [c, n].
  proj matmul + bias.
"""

import numpy as np

import concourse.bass as bass
import concourse.tile as tile
from concourse import bacc, mybir
from concourse._compat import with_exitstack
from concourse.masks import make_identity

B, N, C = 2, 2048, 768
H, D = 12, 64
HALF = 64            # half window
NCORES = 8
CHUNK = 512          # queries per core
NK = CHUNK + 2 * HALF  # 640 rows incl halo
SCALE = D ** -0.5

F16 = mybir.dt.float16
F32 = mybir.dt.float32
IDENT32 = list(range(32))


@with_exitstack
def attn_core_kernel(ctx, tc, outs, ins, repeat=1):
    nc = tc.nc
    out_ap = outs["out"]
    xT, wqkT, wvT, wpT, bias, maskT = (
        ins["xT"], ins["wqkT"], ins["wvT"], ins["wpT"], ins["bias"], ins["maskT"],
    )

    consts = ctx.enter_context(tc.tile_pool(name="consts", bufs=1))
    ppool = ctx.enter_context(tc.tile_pool(name="ps", bufs=2, space="PSUM"))
    scpool = ctx.enter_context(tc.tile_pool(name="scp", bufs=4, space="PSUM"))
    ptpool = ctx.enter_context(tc.tile_pool(name="pt", bufs=18))
    rcpool = ctx.enter_context(tc.tile_pool(name="rc", bufs=4))
    aqpool = ctx.enter_context(tc.tile_pool(name="aq", bufs=2))
    shpool = ctx.enter_context(tc.tile_pool(name="sh", bufs=3))
    outpool = ctx.enter_context(tc.tile_pool(name="ob", bufs=2))

    xT_sb = consts.tile([128, 6, NK], F16)
    wqk_sb = consts.tile([128, 6, 1536], F16)
    wv_sb = consts.tile([128, 6, 768], F16)
    wp_sb = consts.tile([128, 6, 768], F16)
    mask_sb = consts.tile([128, 5, 256], F16)
    bias_sb = consts.tile([128, 768], F32)
    qk_sb = consts.tile([64, 24, NK], F16)      # [d, group, n]; q: h, k: 12+h
    vaug_sb = consts.tile([128, 5, H * 65], F16)  # [n-tile, head*(64+ones)]
    attnT_sb = consts.tile([128, 6, CHUNK], F16)  # [c-tile, n]
    ident_sb = consts.tile([128, 128], F16)
    ones_set = [False]

    # ---- loads ----
    xT3 = xT.rearrange("(t p) n -> p t n", p=128)
    wqk3 = wqkT.rearrange("(t p) e -> p t e", p=128)
    wv3 = wvT.rearrange("(t p) e -> p t e", p=128)
    wp3 = wpT.rearrange("(t p) e -> p t e", p=128)

    def loads():
        # spread input DMAs across the three DMA-capable sequencers so the
        # ~0.5-1.2us per-dma issue cost doesn't serialize (SP alone would
        # take ~20us before the last x tile lands)
        for t in range(6):
            # split the critical qkT inputs across both HWDGE sequencers
            (nc.scalar if t % 2 else nc.sync).dma_start(xT_sb[:, t, :], xT3[:, t, :])
            (nc.sync if t % 2 else nc.scalar).dma_start(wqk_sb[:, t, :], wqk3[:, t, :])
            nc.gpsimd.dma_start(wv_sb[:, t, :], wv3[:, t, :])
            nc.gpsimd.dma_start(wp_sb[:, t, :], wp3[:, t, :])
        for kt in range(5):
            nc.gpsimd.dma_start(mask_sb[:, kt, :], maskT[kt])
        nc.gpsimd.dma_start(bias_sb[:], bias[0:1, :].to_broadcast((128, 768)))
        if not ones_set[0]:
            make_identity(nc, ident_sb[:])
            ones_set[0] = True
        # ones columns of vaug (offset 64, stride 65, 12 per key-tile)
        va = vaug_sb.rearrange("p t (h u) -> p t h u", u=65)
        for kt in range(5):
            nc.vector.memset(va[:, kt, :, 64], 1.0)

    def qkv():
        # qkT: two 64-row head-groups per M=128 matmul
        for pe in range(12):       # pair index: groups 2pe, 2pe+1
            # q (pe<6) only ever read for q_loc in [64, 576)
            chunks = ((64, 512),) if pe < 6 else ((0, 512), (512, 128))
            for c0, w in chunks:
                d0 = c0 - 64 if pe < 6 else c0
                ps = ppool.tile([128, 512], F32, tag="mm")
                for ct in range(6):
                    nc.tensor.matmul(
                        ps[:, :w],
                        wqk_sb[:, ct, pe * 128:(pe + 1) * 128],
                        xT_sb[:, ct, c0:c0 + w],
                        start=(ct == 0), stop=(ct == 5),
                    )
                nc.vector.tensor_copy(out=qk_sb[:, 2 * pe, d0:d0 + w],
                                      in_=ps[0:64, :w])
                sh = shpool.tile([64, 512], F32, tag="sh")
                nc.vector.stream_shuffle(sh[:, :w], ps[64:128, :w], IDENT32)
                nc.scalar.copy(out=qk_sb[:, 2 * pe + 1, d0:d0 + w], in_=sh[:, :w])
        # v -> vaug (strided per-head destination)
        va = vaug_sb.rearrange("p t (h u) -> p t h u", u=65)
        for nt in range(5):
            for c0, w, h0, nh in ((0, 512, 0, 8), (512, 256, 8, 4)):
                ps = ppool.tile([128, 512], F32, tag="mm")
                for ct in range(6):
                    nc.tensor.matmul(
                        ps[:, :w],
                        xT_sb[:, ct, nt * 128:(nt + 1) * 128],
                        wv_sb[:, ct, c0:c0 + w],
                        start=(ct == 0), stop=(ct == 5),
                    )
                nc.vector.tensor_copy(
                    out=va[:, nt, h0:h0 + nh, 0:64],
                    in_=ps[:, :w].rearrange("p (h d) -> p h d", d=64),
                )

    pt_tiles = {}

    def scores_kt(kt):
        # cq range actually consumed downstream; one tile per head PAIR so
        # the exp->mask chain latency per tile is halved and the psum
        # pipeline is 4 tiles deep
        cq0, cq1 = (128, 256) if kt == 0 else ((0, 128) if kt == 4 else (0, 256))
        for hp in range(6):
            sc = scpool.tile([128, 512], F32, tag="sc")
            for j2 in range(2):
                h = 2 * hp + j2
                lhsT = qk_sb[:, 12 + h, kt * 128:kt * 128 + 128]
                rhs = qk_sb[:, h, kt * 128 - 128 + cq0:kt * 128 - 128 + cq1]
                nc.tensor.matmul(sc[:, 256 * j2 + cq0:256 * j2 + cq1], lhsT, rhs,
                                 start=True, stop=True)
            pt = ptpool.tile([128, 512], F16, tag="pt")
            sc2 = sc.rearrange("p (h q) -> p h q", h=2)
            pt2 = pt.rearrange("p (h q) -> p h q", h=2)
            nc.scalar.activation(out=pt2[:, :, cq0:cq1], in_=sc2[:, :, cq0:cq1],
                                 func=mybir.ActivationFunctionType.Exp)
            nc.gpsimd.tensor_tensor(
                pt2[:, :, cq0:cq1], pt2[:, :, cq0:cq1],
                mask_sb[:, kt:kt + 1, cq0:cq1].to_broadcast((128, 2, cq1 - cq0)),
                mybir.AluOpType.mult,
            )
            pt_tiles[(kt, hp)] = pt

    def av_r(r):
        va = vaug_sb.rearrange("p t (h u) -> p t h u", u=65)
        aq = aqpool.tile([128, 768], F16, tag="aq")
        for hg in range(3):
            av = ppool.tile([128, 260], F32, tag="av")
            av3 = av.rearrange("p (h u) -> p h u", u=65)
            for j in range(4):
                h = 4 * hg + j
                for ki, kt in ((0, r), (1, r + 1)):
                    col0 = 128 if ki == 0 else 0
                    pt = pt_tiles[(kt, h // 2)]
                    lhsT = pt[:, 256 * (h % 2) + col0:256 * (h % 2) + col0 + 128]
                    nc.tensor.matmul(av3[:, j, :], lhsT, va[:, kt, h, :],
                                     start=(ki == 0), stop=(ki == 1))
            rc = rcpool.tile([128, 4], F32, tag="rc")
            nc.vector.reciprocal(rc[:], av3[:, :, 64])
            nc.vector.tensor_tensor(
                aq.rearrange("p (h d) -> p h d", d=64)[:, 4 * hg:4 * hg + 4, :],
                av3[:, :, 0:64],
                rc[:, :, None].to_broadcast((128, 4, 64)),
                mybir.AluOpType.mult,
            )
        # transpose [q, c] -> attnT [c, q] per head pair; one batched copy
        qsl = slice(128 * r, 128 * r + 128)
        tr = ppool.tile([128, 6, 128], F16, tag="mm")
        for hp in range(6):
            nc.tensor.transpose(tr[:, hp, :], aq[:, 128 * hp:128 * hp + 128],
                                ident_sb[:])
        nc.vector.tensor_copy(out=attnT_sb[:, :, qsl], in_=tr[:])

    def proj_r(r):
        ob = outpool.tile([128, 768], F32, tag="ob")
        for c0, w in ((0, 512), (512, 256)):
            ps = ppool.tile([128, 512], F32, tag="mm")
            for ct in range(6):
                nc.tensor.matmul(
                    ps[:, :w],
                    attnT_sb[:, ct, 128 * r:128 * r + 128],
                    wp_sb[:, ct, c0:c0 + w],
                    start=(ct == 0), stop=(ct == 5),
                )
            nc.vector.tensor_add(out=ob[:, c0:c0 + w], in0=ps[:, :w],
                                 in1=bias_sb[:, c0:c0 + w])
        nc.sync.dma_start(out_ap[128 * r:128 * r + 128, :], ob[:])

    for _rep in range(repeat):
        pt_tiles.clear()
        loads()
        qkv()
        # software-pipelined: proj of round r-1 is emitted between the
        # next scores and av so PE has dependency-free work while the
        # exp->mask chain for round r completes
        scores_kt(0)
        scores_kt(1)
        av_r(0)
        for r in range(1, 4):
            scores_kt(r + 1)
            proj_r(r - 1)
            av_r(r)
        proj_r(3)


def build_nc(repeat=1):
    nc = bacc.Bacc("TRN2", target_bir_lowering=False, debug=False)
    ins = {
        "xT": nc.dram_tensor("xT", [C, NK], F16, kind="ExternalInput").ap(),
        "wqkT": nc.dram_tensor("wqkT", [C, 2 * C], F16, kind="ExternalInput").ap(),
        "wvT": nc.dram_tensor("wvT", [C, C], F16, kind="ExternalInput").ap(),
        "wpT": nc.dram_tensor("wpT", [C, C], F16, kind="ExternalInput").ap(),
        "bias": nc.dram_tensor("bias", [1, C], F32, kind="ExternalInput").ap(),
        "maskT": nc.dram_tensor("maskT", [5, 128, 256], F16, kind="ExternalInput").ap(),
    }
    outs = {"out": nc.dram_tensor("out", [CHUNK, C], F32, kind="ExternalOutput").ap()}
    with tile.TileContext(nc) as tc:
        attn_core_kernel(tc, outs, ins, repeat=repeat)
    nc.finalize()
    return nc


def make_core_inputs(x, w_qkv, w_proj, b_proj):
    """Build the 8 per-core input maps from full inputs."""
    x = np.asarray(x, dtype=np.float32)
    w_qkv = np.asarray(w_qkv, dtype=np.float32)
    w_proj = np.asarray(w_proj, dtype=np.float32)
    b_proj = np.asarray(b_proj, dtype=np.float32)

    wqk = np.concatenate([w_qkv[:C] * SCALE, w_qkv[C:2 * C]], axis=0)
    wqkT = np.ascontiguousarray(wqk.T).astype(np.float16)
    wvT = np.ascontiguousarray(w_qkv[2 * C:].T).astype(np.float16)
    wpT = np.ascontiguousarray(w_proj.T).astype(np.float16)
    bias = b_proj.reshape(1, C).astype(np.float32)

    in_maps = []
    for c in range(NCORES):
        b, s = divmod(c, 4)
        lo = s * CHUNK - HALF
        hi = s * CHUNK + CHUNK + HALF
        xs = np.zeros((NK, C), dtype=np.float32)
        s0, s1 = max(lo, 0), min(hi, N)
        xs[s0 - lo:s1 - lo] = x[b, s0:s1]
        xT = np.ascontiguousarray(xs.T).astype(np.float16)

        mask = np.zeros((5, 128, 256), dtype=np.float16)
        k = np.arange(128)[:, None]
        cq = np.arange(256)[None, :]
        band = (cq - k >= 0) & (cq - k <= 128)
        for kt in range(5):
            key_seq = s * CHUNK - HALF + 128 * kt + k
            valid = (key_seq >= 0) & (key_seq < N)
            mask[kt] = (band & valid).astype(np.float16)

        in_maps.append({
            "xT": xT, "wqkT": wqkT, "wvT": wvT, "wpT": wpT,
            "bias": bias, "maskT": mask,
        })
    return in_maps


_NC_CACHE = None


def kernel(x, w_qkv, w_proj, b_proj):
    from concourse.bass_utils import run_bass_kernel_spmd

    global _NC_CACHE
    if _NC_CACHE is None:
        _NC_CACHE = build_nc()
    in_maps = make_core_inputs(x, w_qkv, w_proj, b_proj)
    res = run_bass_kernel_spmd(_NC_CACHE, in_maps, core_ids=list(range(NCORES)))
    out = np.empty((B, N, C), dtype=np.float32)
    for c in range(NCORES):
        b, s = divmod(c, 4)
        out[b, s * CHUNK:(s + 1) * CHUNK] = res.results[c]["out"]
    return out


# revision 26
# speedup vs baseline: 1.0229x; 1.0229x over previous
"""Sliding-window attention (WINDOW=129) Trainium2 Bass kernel.

Problem: x[B=2, N=2048, C=768] -> qkv proj -> 12-head sliding-window
attention (half-window 64) -> output proj + bias.

Sharding: sequence-parallel over 8 cores: core c handles batch b = c//4,
query chunk s = c%4 (512 queries), with a 64-row halo each side for K/V.
Weights replicated. Each core computes its 512 output rows completely;
host concatenates. No collectives.

Per-core pipeline (matmul operands fp16, psum f32):
  qkT [e, n] via M=128 matmuls (two 64-row head-groups per psum); the
    upper psum half moves to partition base 0 via stream_shuffle (matmul
    operands at partition base 64 fault this device) + ACT copy.
  v -> vaug [n, 12*(64+1)] with a fused ones column per head.
  scores sT[k, q] per key-tile via K=64 matmuls; exp on ACT (scores are
    N(0,1)-scale so no max subtraction); 0/1 band/validity mask on DVE.
  AV with q on the output partition dim: out[q, 65] = pT_slice.T @ vaug;
    column 64 gives the softmax denominator; reciprocal + per-partition
    broadcast multiply normalizes; PE transpose -> attnT [c, n].
  proj matmul + bias.
"""

import numpy as np

import concourse.bass as bass
import concourse.tile as tile
from concourse import bacc, mybir
from concourse._compat import with_exitstack
from concourse.masks import make_identity

B, N, C = 2, 2048, 768
H, D = 12, 64
HALF = 64            # half window
NCORES = 8
CHUNK = 512          # queries per core
NK = CHUNK + 2 * HALF  # 640 rows incl halo
SCALE = D ** -0.5

F16 = mybir.dt.float16
F32 = mybir.dt.float32
IDENT32 = list(range(32))


@with_exitstack
def attn_core_kernel(ctx, tc, outs, ins, repeat=1):
    nc = tc.nc
    out_ap = outs["out"]
    xT, wqkT, wvT, wpT, bias, maskT = (
        ins["xT"], ins["wqkT"], ins["wvT"], ins["wpT"], ins["bias"], ins["maskT"],
    )

    consts = ctx.enter_context(tc.tile_pool(name="consts", bufs=1))
    ppool = ctx.enter_context(tc.tile_pool(name="ps", bufs=2, space="PSUM"))
    scpool = ctx.enter_context(tc.tile_pool(name="scp", bufs=4, space="PSUM"))
    ptpool = ctx.enter_context(tc.tile_pool(name="pt", bufs=18))
    rcpool = ctx.enter_context(tc.tile_pool(name="rc", bufs=4))
    aqpool = ctx.enter_context(tc.tile_pool(name="aq", bufs=2))
    shpool = ctx.enter_context(tc.tile_pool(name="sh", bufs=3))
    outpool = ctx.enter_context(tc.tile_pool(name="ob", bufs=2))

    xT_sb = consts.tile([128, 6, NK], F16)
    wqk_sb = consts.tile([128, 6, 1536], F16)
    wv_sb = consts.tile([128, 6, 768], F16)
    wp_sb = consts.tile([128, 6, 768], F16)
    mask_sb = consts.tile([128, 5, 256], F16)
    bias_sb = consts.tile([128, 768], F32)
    qk_sb = consts.tile([64, 24, NK], F16)      # [d, group, n]; q: h, k: 12+h
    vaug_sb = consts.tile([128, 5, H * 65], F16)  # [n-tile, head*(64+ones)]
    attnT_sb = consts.tile([128, 6, CHUNK], F16)  # [c-tile, n]
    ident_sb = consts.tile([128, 128], F16)
    ones_set = [False]

    # ---- loads ----
    xT3 = xT.rearrange("(t p) n -> p t n", p=128)
    wqk3 = wqkT.rearrange("(t p) e -> p t e", p=128)
    wv3 = wvT.rearrange("(t p) e -> p t e", p=128)
    wp3 = wpT.rearrange("(t p) e -> p t e", p=128)

    def loads():
        # spread input DMAs across the three DMA-capable sequencers so the
        # ~0.5-1.2us per-dma issue cost doesn't serialize (SP alone would
        # take ~20us before the last x tile lands)
        # tiny head-pieces first so pair 0's first accumulation can start
        # ~2us earlier than the full-tile transfers allow
        nc.sync.dma_start(wqk_sb[:, 0, 0:128], wqk3[:, 0, 0:128])
        nc.scalar.dma_start(xT_sb[:, 0, 64:576], xT3[:, 0, 64:576])
        for t in range(6):
            # split the critical qkT inputs across both HWDGE sequencers
            if t == 0:
                nc.scalar.dma_start(xT_sb[:, 0, 0:64], xT3[:, 0, 0:64])
                nc.scalar.dma_start(xT_sb[:, 0, 576:640], xT3[:, 0, 576:640])
                nc.sync.dma_start(wqk_sb[:, 0, 128:1536], wqk3[:, 0, 128:1536])
            else:
                (nc.scalar if t % 2 else nc.sync).dma_start(xT_sb[:, t, :], xT3[:, t, :])
                (nc.sync if t % 2 else nc.scalar).dma_start(wqk_sb[:, t, :], wqk3[:, t, :])
            nc.gpsimd.dma_start(wv_sb[:, t, :], wv3[:, t, :])
            nc.gpsimd.dma_start(wp_sb[:, t, :], wp3[:, t, :])
        for kt in range(5):
            nc.gpsimd.dma_start(mask_sb[:, kt, :], maskT[kt])
        nc.gpsimd.dma_start(bias_sb[:], bias[0:1, :].to_broadcast((128, 768)))
        if not ones_set[0]:
            make_identity(nc, ident_sb[:])
            ones_set[0] = True
        # ones columns of vaug (offset 64, stride 65, 12 per key-tile)
        va = vaug_sb.rearrange("p t (h u) -> p t h u", u=65)
        for kt in range(5):
            nc.vector.memset(va[:, kt, :, 64], 1.0)

    def qkv():
        # qkT: two 64-row head-groups per M=128 matmul
        for pe in range(12):       # pair index: groups 2pe, 2pe+1
            # q (pe<6) only ever read for q_loc in [64, 576)
            chunks = ((64, 512),) if pe < 6 else ((0, 512), (512, 128))
            for c0, w in chunks:
                d0 = c0 - 64 if pe < 6 else c0
                ps = ppool.tile([128, 512], F32, tag="mm")
                for ct in range(6):
                    nc.tensor.matmul(
                        ps[:, :w],
                        wqk_sb[:, ct, pe * 128:(pe + 1) * 128],
                        xT_sb[:, ct, c0:c0 + w],
                        start=(ct == 0), stop=(ct == 5),
                    )
                nc.vector.tensor_copy(out=qk_sb[:, 2 * pe, d0:d0 + w],
                                      in_=ps[0:64, :w])
                sh = shpool.tile([64, 512], F32, tag="sh")
                nc.vector.stream_shuffle(sh[:, :w], ps[64:128, :w], IDENT32)
                nc.scalar.copy(out=qk_sb[:, 2 * pe + 1, d0:d0 + w], in_=sh[:, :w])
        # v -> vaug (strided per-head destination)
        va = vaug_sb.rearrange("p t (h u) -> p t h u", u=65)
        for nt in range(5):
            for c0, w, h0, nh in ((0, 512, 0, 8), (512, 256, 8, 4)):
                ps = ppool.tile([128, 512], F32, tag="mm")
                for ct in range(6):
                    nc.tensor.matmul(
                        ps[:, :w],
                        xT_sb[:, ct, nt * 128:(nt + 1) * 128],
                        wv_sb[:, ct, c0:c0 + w],
                        start=(ct == 0), stop=(ct == 5),
                    )
                nc.vector.tensor_copy(
                    out=va[:, nt, h0:h0 + nh, 0:64],
                    in_=ps[:, :w].rearrange("p (h d) -> p h d", d=64),
                )

    pt_tiles = {}

    def scores_kt(kt):
        # cq range actually consumed downstream; one tile per head PAIR so
        # the exp->mask chain latency per tile is halved and the psum
        # pipeline is 4 tiles deep
        cq0, cq1 = (128, 256) if kt == 0 else ((0, 128) if kt == 4 else (0, 256))
        for hp in range(6):
            sc = scpool.tile([128, 512], F32, tag="sc")
            for j2 in range(2):
                h = 2 * hp + j2
                lhsT = qk_sb[:, 12 + h, kt * 128:kt * 128 + 128]
                rhs = qk_sb[:, h, kt * 128 - 128 + cq0:kt * 128 - 128 + cq1]
                nc.tensor.matmul(sc[:, 256 * j2 + cq0:256 * j2 + cq1], lhsT, rhs,
                                 start=True, stop=True)
            pt = ptpool.tile([128, 512], F16, tag="pt")
            sc2 = sc.rearrange("p (h q) -> p h q", h=2)
            pt2 = pt.rearrange("p (h q) -> p h q", h=2)
            nc.scalar.activation(out=pt2[:, :, cq0:cq1], in_=sc2[:, :, cq0:cq1],
                                 func=mybir.ActivationFunctionType.Exp)
            nc.gpsimd.tensor_tensor(
                pt2[:, :, cq0:cq1], pt2[:, :, cq0:cq1],
                mask_sb[:, kt:kt + 1, cq0:cq1].to_broadcast((128, 2, cq1 - cq0)),
                mybir.AluOpType.mult,
            )
            pt_tiles[(kt, hp)] = pt

    def av_r(r):
        va = vaug_sb.rearrange("p t (h u) -> p t h u", u=65)
        aq = aqpool.tile([128, 768], F16, tag="aq")
        for hg in range(3):
            av = ppool.tile([128, 260], F32, tag="av")
            av3 = av.rearrange("p (h u) -> p h u", u=65)
            for j in range(4):
                h = 4 * hg + j
                for ki, kt in ((0, r), (1, r + 1)):
                    col0 = 128 if ki == 0 else 0
                    pt = pt_tiles[(kt, h // 2)]
                    lhsT = pt[:, 256 * (h % 2) + col0:256 * (h % 2) + col0 + 128]
                    nc.tensor.matmul(av3[:, j, :], lhsT, va[:, kt, h, :],
                                     start=(ki == 0), stop=(ki == 1))
            rc = rcpool.tile([128, 4], F32, tag="rc")
            nc.vector.reciprocal(rc[:], av3[:, :, 64])
            nc.vector.tensor_tensor(
                aq.rearrange("p (h d) -> p h d", d=64)[:, 4 * hg:4 * hg + 4, :],
                av3[:, :, 0:64],
                rc[:, :, None].to_broadcast((128, 4, 64)),
                mybir.AluOpType.mult,
            )
        # transpose [q, c] -> attnT [c, q] per head pair; one batched copy
        qsl = slice(128 * r, 128 * r + 128)
        tr = ppool.tile([128, 6, 128], F16, tag="mm")
        for hp in range(6):
            nc.tensor.transpose(tr[:, hp, :], aq[:, 128 * hp:128 * hp + 128],
                                ident_sb[:])
        nc.vector.tensor_copy(out=attnT_sb[:, :, qsl], in_=tr[:])

    def proj_r(r):
        ob = outpool.tile([128, 768], F32, tag="ob")
        for c0, w in ((0, 512), (512, 256)):
            ps = ppool.tile([128, 512], F32, tag="mm")
            for ct in range(6):
                nc.tensor.matmul(
                    ps[:, :w],
                    attnT_sb[:, ct, 128 * r:128 * r + 128],
                    wp_sb[:, ct, c0:c0 + w],
                    start=(ct == 0), stop=(ct == 5),
                )
            nc.vector.tensor_add(out=ob[:, c0:c0 + w], in0=ps[:, :w],
                                 in1=bias_sb[:, c0:c0 + w])
        nc.sync.dma_start(out_ap[128 * r:128 * r + 128, :], ob[:])

    for _rep in range(repeat):
        pt_tiles.clear()
        loads()
        qkv()
        # software-pipelined: proj of round r-1 is emitted between the
        # next scores and av so PE has dependency-free work while the
        # exp->mask chain for round r completes
        scores_kt(0)
        scores_kt(1)
        av_r(0)
        for r in range(1, 4):
            scores_kt(r + 1)
            proj_r(r - 1)
            av_r(r)
        proj_r(3)


def build_nc(repeat=1):
    nc = bacc.Bacc("TRN2", target_bir_lowering=False, debug=False)
    ins = {
        "xT": nc.dram_tensor("xT", [C, NK], F16, kind="ExternalInput").ap(),
        "wqkT": nc.dram_tensor("wqkT", [C, 2 * C], F16, kind="ExternalInput").ap(),
        "wvT": nc.dram_tensor("wvT", [C, C], F16, kind="ExternalInput").ap(),
        "wpT": nc.dram_tensor("wpT", [C, C], F16, kind="ExternalInput").ap(),
        "bias": nc.dram_tensor("bias", [1, C], F32, kind="ExternalInput").ap(),
        "maskT": nc.dram_tensor("maskT", [5, 128, 256], F16, kind="ExternalInput").ap(),
    }
    outs = {"out": nc.dram_tensor("out", [CHUNK, C], F32, kind="ExternalOutput").ap()}
    with tile.TileContext(nc) as tc:
        attn_core_kernel(tc, outs, ins, repeat=repeat)
    nc.finalize()
    return nc


def make_core_inputs(x, w_qkv, w_proj, b_proj):
    """Build the 8 per-core input maps from full inputs."""
    x = np.asarray(x, dtype=np.float32)
    w_qkv = np.asarray(w_qkv, dtype=np.float32)
    w_proj = np.asarray(w_proj, dtype=np.float32)
    b_proj = np.asarray(b_proj, dtype=np.float32)

    wqk = np.concatenate([w_qkv[:C] * SCALE, w_qkv[C:2 * C]], axis=0)
    wqkT = np.ascontiguousarray(wqk.T).astype(np.float16)
    wvT = np.ascontiguousarray(w_qkv[2 * C:].T).astype(np.float16)
    wpT = np.ascontiguousarray(w_proj.T).astype(np.float16)
    bias = b_proj.reshape(1, C).astype(np.float32)

    in_maps = []
    for c in range(NCORES):
        b, s = divmod(c, 4)
        lo = s * CHUNK - HALF
        hi = s * CHUNK + CHUNK + HALF
        xs = np.zeros((NK, C), dtype=np.float32)
        s0, s1 = max(lo, 0), min(hi, N)
        xs[s0 - lo:s1 - lo] = x[b, s0:s1]
        xT = np.ascontiguousarray(xs.T).astype(np.float16)

        mask = np.zeros((5, 128, 256), dtype=np.float16)
        k = np.arange(128)[:, None]
        cq = np.arange(256)[None, :]
        band = (cq - k >= 0) & (cq - k <= 128)
        for kt in range(5):
            key_seq = s * CHUNK - HALF + 128 * kt + k
            valid = (key_seq >= 0) & (key_seq < N)
            mask[kt] = (band & valid).astype(np.float16)

        in_maps.append({
            "xT": xT, "wqkT": wqkT, "wvT": wvT, "wpT": wpT,
            "bias": bias, "maskT": mask,
        })
    return in_maps


_NC_CACHE = None


def kernel(x, w_qkv, w_proj, b_proj):
    from concourse.bass_utils import run_bass_kernel_spmd

    global _NC_CACHE
    if _NC_CACHE is None:
        _NC_CACHE = build_nc()
    in_maps = make_core_inputs(x, w_qkv, w_proj, b_proj)
    res = run_bass_kernel_spmd(_NC_CACHE, in_maps, core_ids=list(range(NCORES)))
    out = np.empty((B, N, C), dtype=np.float32)
    for c in range(NCORES):
        b, s = divmod(c, 4)
        out[b, s * CHUNK:(s + 1) * CHUNK] = res.results[c]["out"]
    return out
